# revision 26
# baseline (speedup 1.0000x reference)
"""Multi-head causal self-attention (B=4, T=2048, C=1024, 16 heads) on 8 trn2 cores.

Sharding: data-parallel over batch (4) x tensor-parallel over heads (2 groups of 8).
Core m handles batch m//2, head group m%2. Host pre-transposes x and the weights so
every on-device matmul consumes operands in natural layout (zero on-device
transposes); the output projection partial sums are pair-reduced on host (+bias).

Per-core pipeline (all matmuls fp32r = FP22 multiply, fp32 PSUM accumulate):
  qT[o,t] = Wq_g @ x^T        (lhsT = Wq_g^T chunks, rhs = x^T chunks)
  kT[o,t] likewise; v[t,o]    (lhsT = x^T chunks, rhs = Wv_g^T)
  scores^T[k,q] per head      (lhsT = kT tile [64,128], rhs = qT tile [64,512])
  p = exp(0.125*scores^T)     (ACT, causal mask via memset + triangle multiply)
  [AV^T | denom] = [v|1]^T @ p (ones column of v gives softmax denominators)
  avT = AV^T * exp(-ln(denom)) broadcast via K=1 ones-matmul
  out_partial = avT^T @ Wp_g^T
"""

import numpy as np

import concourse.bass as bass
import concourse.mybir as mybir
import concourse.tile as tile
from concourse.bass_utils import run_bass_kernel_spmd

F32 = mybir.dt.float32
F32R = mybir.dt.float32r
AF = mybir.ActivationFunctionType
MULT = mybir.AluOpType.mult

B, T, C = 4, 2048, 1024
HEADS, D = 16, 64
GROUPS = 2                  # head groups (tensor parallel)
HPC = HEADS // GROUPS       # heads per core = 8
GC = HPC * D                # group channel width = 512
NKC = T // 128              # Tk chunks = 16
NJ = T // 512               # Tq tiles = 4
CCH = C // 128              # contraction chunks = 8
NSTRIP = T // 512           # phase-1 t strips = 4

_PROGRAM = None


def _patch_drain_chunking():
    """The axon walrus build rejects instructions with >~4 sem waits; Tile's
    kernel-tail drain waits on every live semaphore at once. Split it into a
    chain of drains with <=2 waits each."""
    from bass_rust import VectorClock, ScopedClock

    if getattr(tile.TileContext, "_drain_chunk_patched", False):
        return

    def _drain_and_barrier(self, tick_clock, wait_clock):
        gc_vec = list(tick_clock.global_clock)
        nz = [i for i, t in enumerate(gc_vec) if t > 0]
        CHUNK = 1
        for k in range(0, len(nz), CHUNK):
            keep = set(nz[k:k + CHUNK])
            partial = [gc_vec[i] if i in keep else 0 for i in range(len(gc_vec))]
            d = self.nc.sync.drain()
            wait_clock.add_sem_waits(d.ins, ScopedClock({None: VectorClock(partial)}))
        self.nc.all_engine_barrier()
        assert self.sems is not None
        popped = self.nc._tile_sem_poison_stack.pop()
        assert popped is self._sem_poison
        self.nc.clear_and_free_semaphores(list(self.sems.allocated().values()))
        self.nc.all_engine_barrier()

    tile.TileContext._drain_and_barrier = _drain_and_barrier
    tile.TileContext._drain_chunk_patched = True


def _split_excess_waits(nc, maxw=1, maxw_other=None):
    """Walrus rejects instructions carrying more than ~1 sem wait (proven for
    PE matmul S3_LW and the SP drain at 5). Move excess waits onto same-engine
    NoOps inserted immediately before the instruction (engine streams execute
    in bb order, so semantics are preserved). maxw_other, if set, applies to
    non-PE engines."""
    from bass_rust import InstNoOp

    ctr = 0
    for f in nc.m.functions:
        for bb in f.blocks:
            new_insts = []
            for inst in bb.instructions:
                si = inst.sync_info
                waits = list(si.on_wait) if si and si.on_wait else []
                lim = maxw
                if maxw_other is not None and str(inst.engine) != 'EngineType.PE':
                    lim = maxw_other
                maxw_eff = lim
                if len(waits) > maxw_eff:
                    head, rest = waits[:-maxw_eff], waits[-maxw_eff:]
                    for k in range(0, len(head), maxw_eff):
                        ctr += 1
                        new_insts.append(InstNoOp(
                            name=f"waitnop_{ctr}",
                            engine=inst.engine,
                            sync_info=mybir.SyncInfo(
                                on_wait=head[k:k + maxw_eff], on_update=[]),
                        ))
                    inst.sync_info = mybir.SyncInfo(on_wait=rest, on_update=si.on_update)
                new_insts.append(inst)
            bb.instructions = new_insts
    return ctr


def _build_program():
    _patch_drain_chunking()
    nc = bass.Bass()

    xT_d = nc.declare_dram_parameter("xT", [C, T], F32R, isOutput=False)
    wq_d = nc.declare_dram_parameter("wqT", [C, GC], F32R, isOutput=False)
    wk_d = nc.declare_dram_parameter("wkT", [C, GC], F32R, isOutput=False)
    wv_d = nc.declare_dram_parameter("wvT", [C, GC], F32R, isOutput=False)
    wp_d = nc.declare_dram_parameter("wpT", [GC, C], F32R, isOutput=False)
    out_d = nc.declare_dram_parameter("outp", [T, C], F32, isOutput=True)

    from contextlib import ExitStack

    with tile.TileContext(nc) as tc, ExitStack() as stack:
        cpool = stack.enter_context(tc.tile_pool(name="const", bufs=1))
        qkv_pool = stack.enter_context(tc.tile_pool(name="qkv", bufs=1))

        # additive causal mask: 0 where q >= k, -1e9 where q < k (exp -> 0)
        maskneg = cpool.tile([128, 128], F32)
        nc.gpsimd.memset(maskneg[:, :], 0.0)
        nc.gpsimd.affine_select(
            out=maskneg[:, :], in_=maskneg[:, :],
            compare_op=mybir.AluOpType.is_ge, fill=-1e9, base=0,
            pattern=[[1, 128]], channel_multiplier=-1,
        )
        # DVE cannot encode f32r, so f32r tiles are written by ACT/DMA only
        ones = cpool.tile([128, 128], F32R)
        nc.scalar.activation(ones[64:65, :], ones[64:65, :], AF.Copy, scale=0.0, bias=1.0)

        qT = qkv_pool.tile([128, HPC // 2, T], F32R)   # [c, head-pair, t]
        kT = qkv_pool.tile([128, HPC // 2, T], F32R)
        # v padded with a ones column per head: [t-chunk, head, 65]
        v = qkv_pool.tile([128, NKC, HPC, D + 1], F32R)
        nc.scalar.activation(v[:, :, :, D:D + 1], v[:, :, :, D:D + 1],
                             AF.Copy, scale=0.0, bias=1.0)

        # ---------------- Phase 1: QKV projections ----------------
        with tc.tile_pool(name="w1", bufs=1) as wpool, \
             tc.tile_pool(name="xs", bufs=2) as xpool, \
             tc.tile_pool(name="tmp1", bufs=4) as tmp1, \
             tc.tile_pool(name="ps1", bufs=8, space="PSUM") as ps1:
            wq = wpool.tile([128, CCH, GC], F32R)
            wk = wpool.tile([128, CCH, GC], F32R)
            wv = wpool.tile([128, CCH, GC], F32R)

            for s in range(NSTRIP):
                xs = xpool.tile([128, CCH, 512], F32R)
                nc.sync.dma_start(
                    xs[:, :, :],
                    xT_d[:, 512 * s:512 * (s + 1)].rearrange("(c p) t -> p c t", p=128))
                if s == 0:
                    # batched weight loads, emitted after the first x strip so
                    # the PE can start as soon as wq lands (wq first: q runs first)
                    for w_sb, w_d in ((wq, wq_d), (wk, wk_d), (wv, wv_d)):
                        nc.sync.dma_start(w_sb[:, :, :],
                                          w_d[:, :].rearrange("(c p) o -> p c o", p=128))
                for w_sb, dst in ((wq, qT), (wk, kT)):
                    for o in range(HPC // 2):
                        pq = ps1.tile([128, 512], F32, tag="pp")
                        for c in range(CCH):
                            nc.tensor.matmul(pq[:, :], w_sb[:, c, 128 * o:128 * (o + 1)],
                                             xs[:, c, :], start=(c == 0), stop=(c == CCH - 1))
                        tq = tmp1.tile([128, 512], F32, tag="t1")
                        nc.vector.tensor_copy(tq[:, :], pq[:, :])
                        nc.sync.dma_start(dst[:, o, 512 * s:512 * (s + 1)],
                                          tq[:, :].bitcast(F32R))
                for tt in range(4):
                    pv = ps1.tile([128, 512], F32, tag="pp")
                    for c in range(CCH):
                        nc.tensor.matmul(pv[:, :], xs[:, c, 128 * tt:128 * (tt + 1)],
                                         wv[:, c, :], start=(c == 0), stop=(c == CCH - 1))
                    tv = tmp1.tile([128, 512], F32, tag="t1")
                    nc.vector.tensor_copy(tv[:, :], pv[:, :])
                    nc.sync.dma_start(
                        v[:, 4 * s + tt, :, 0:D],
                        tv[:, :].rearrange("p (h d) -> p h d", h=HPC).bitcast(F32R))

        # ---------------- Phase 2+3: attention + output projection ----------------
        avT = stack.enter_context(tc.tile_pool(name="avt", bufs=1)).tile([128, HPC // 2, T], F32R)
        wp = stack.enter_context(tc.tile_pool(name="wp", bufs=1)).tile([128, GC // 128, C], F32R)
        nc.sync.dma_start(wp[:, :, :], wp_d[:, :].rearrange("(c p) o -> p c o", p=128))

        with tc.tile_pool(name="pt", bufs=8) as pt_pool, \
             tc.tile_pool(name="dd", bufs=4) as d_pool, \
             tc.tile_pool(name="rr", bufs=3) as r_pool, \
             tc.tile_pool(name="avtmp", bufs=3) as avtmp_pool, \
             tc.tile_pool(name="ob", bufs=4) as out_pool, \
             tc.tile_pool(name="ps_s", bufs=4, space="PSUM") as ps_s, \
             tc.tile_pool(name="ps_av", bufs=2, space="PSUM") as ps_av, \
             tc.tile_pool(name="ps_bc", bufs=1, space="PSUM") as ps_bc, \
             tc.tile_pool(name="ps_o", bufs=1, space="PSUM") as ps_o:

            for j in range(NJ):
                for hp in range(HPC // 2):
                    nkc = 4 * (j + 1)
                    # both heads of the pair run interleaved: their scores
                    # matmuls sit in adjacent PE slots with disjoint row
                    # groups (K=64 at partition 0 vs 64) and overlap on HW
                    av0 = ps_av.tile([65, 512], F32, tag="av")
                    av1 = ps_av.tile([65, 512], F32, tag="av")
                    avs = [av0, av1]
                    def emit_scores_exp(i):
                        out = []
                        for par in range(2):
                            pb = 64 * par
                            sps = ps_s.tile([128, 512], F32, tag="s")
                            nc.tensor.matmul(
                                sps[:, :],
                                kT[pb:pb + 64, hp, 128 * i:128 * (i + 1)],
                                qT[pb:pb + 64, hp, 512 * j:512 * (j + 1)],
                                start=True, stop=True)
                            ptile = pt_pool.tile([128, 512], F32R, tag="pt")
                            roff = 128 * i - 512 * j
                            if roff >= 0:
                                # diagonal tile: add -1e9 above the diagonal in
                                # PSUM, then exp only the columns [roff:512] the
                                # AV matmul will consume (cols [0:roff] are
                                # fully masked and skipped outright)
                                nc.vector.tensor_tensor(
                                    sps[:, roff:roff + 128], sps[:, roff:roff + 128],
                                    maskneg[:, :], op=mybir.AluOpType.add)
                                nc.scalar.activation(ptile[:, roff:512], sps[:, roff:512],
                                                     AF.Exp, scale=0.125)
                            else:
                                roff = 0
                                nc.scalar.activation(ptile[:, :], sps[:, :], AF.Exp, scale=0.125)
                            out.append((ptile, roff))
                        return out

                    def emit_av(i, pts):
                        for par in range(2):
                            ptile, roff = pts[par]
                            nc.tensor.matmul(avs[par][:, roff:512], v[:, i, 2 * hp + par, :],
                                             ptile[:, roff:512],
                                             start=(i == 0), stop=(i == nkc - 1))

                    # one-chunk software pipeline: chunk i+1's scores sit ahead
                    # of chunk i's AV matmuls in the PE stream, so AV never
                    # waits out the exp latency
                    prev = emit_scores_exp(0)
                    for i in range(1, nkc):
                        cur = emit_scores_exp(i)
                        emit_av(i - 1, prev)
                        prev = cur
                    emit_av(nkc - 1, prev)
                    for par in range(2):
                        av = avs[par]
                        # single DVE copy frees the AV PSUM bank immediately so
                        # the next head pair's AV matmuls are not gated on the
                        # whole normalize chain
                        avr = avtmp_pool.tile([65, 512], F32, tag="avr")
                        nc.vector.tensor_copy(avr[:, :], av[:, :])
                        # softmax denominators: r = exp(-ln(denom)), broadcast via K=1 matmul
                        dt_ = d_pool.tile([65, 512], F32R, tag="d")
                        nc.scalar.activation(dt_[64:65, :], avr[64:65, :], AF.Ln)
                        nc.scalar.activation(dt_[64:65, :], dt_[64:65, :], AF.Exp, scale=-1.0)
                        bc = ps_bc.tile([128, 512], F32, tag="bc")
                        nc.tensor.matmul(bc[:, :], ones[64:65, :], dt_[64:65, :],
                                         start=True, stop=True)
                        rb = r_pool.tile([64, 512], F32, tag="r")
                        nc.vector.tensor_copy(rb[:, :], bc[0:64, :])
                        avf = avtmp_pool.tile([64, 512], F32, tag="avf")
                        nc.vector.tensor_tensor(avf[:, :], avr[0:64, :], rb[:, :], op=MULT)
                        # DMA moves lanes 0:64 to the destination partitions
                        nc.sync.dma_start(avT[64 * par:64 * par + 64, hp, 512 * j:512 * (j + 1)],
                                          avf[:, :].bitcast(F32R))

                # output projection for the t-tiles whose avT columns just completed
                for tt in range(4 * j, 4 * (j + 1)):
                    ob = out_pool.tile([128, C], F32, tag="ob")
                    for o2 in range(2):
                        po = ps_o.tile([128, 512], F32, tag="o")
                        for c4 in range(GC // 128):
                            nc.tensor.matmul(po[:, :], avT[:, c4, 128 * tt:128 * (tt + 1)],
                                             wp[:, c4, 512 * o2:512 * (o2 + 1)],
                                             start=(c4 == 0), stop=(c4 == GC // 128 - 1))
                        nc.vector.tensor_copy(ob[:, 512 * o2:512 * (o2 + 1)], po[:, :])
                    nc.sync.dma_start(out_d[128 * tt:128 * (tt + 1), :], ob[:, :])
    _split_excess_waits(nc)
    return nc


def _get_program():
    global _PROGRAM
    if _PROGRAM is None:
        _PROGRAM = _build_program()
    return _PROGRAM


def _make_in_maps(x, Wk, Wq, Wv, Wp):
    x = np.asarray(x, dtype=np.float32)
    Wk = np.asarray(Wk, dtype=np.float32)
    Wq = np.asarray(Wq, dtype=np.float32)
    Wv = np.asarray(Wv, dtype=np.float32)
    Wp = np.asarray(Wp, dtype=np.float32)
    in_maps = []
    for core in range(8):
        b, g = core // GROUPS, core % GROUPS
        rows = slice(GC * g, GC * (g + 1))
        in_maps.append({
            "xT": np.ascontiguousarray(x[b].T),                 # [C, T]
            "wqT": np.ascontiguousarray(Wq[rows, :].T),         # [C, GC]
            "wkT": np.ascontiguousarray(Wk[rows, :].T),
            "wvT": np.ascontiguousarray(Wv[rows, :].T),
            "wpT": np.ascontiguousarray(Wp[:, rows].T),         # [GC, C]
        })
    return in_maps


def run(x, Wk, Wq, Wv, Wp, bp, trace=False, **spmd_kwargs):
    nc = _get_program()
    in_maps = _make_in_maps(x, Wk, Wq, Wv, Wp)
    res = run_bass_kernel_spmd(nc, in_maps, list(range(8)), trace=trace, **spmd_kwargs)
    bp = np.asarray(bp, dtype=np.float32)
    out = np.empty((B, T, C), dtype=np.float32)
    for b in range(B):
        out[b] = res.results[GROUPS * b]["outp"] + res.results[GROUPS * b + 1]["outp"] + bp
    return out, res


def kernel(x, Wk, Wq, Wv, Wp, bp):
    out, _ = run(x, Wk, Wq, Wv, Wp, bp)
    return out


# revision 27
# speedup vs baseline: 1.0013x; 1.0013x over previous
"""Multi-head causal self-attention (B=4, T=2048, C=1024, 16 heads) on 8 trn2 cores.

Sharding: data-parallel over batch (4) x tensor-parallel over heads (2 groups of 8).
Core m handles batch m//2, head group m%2. Host pre-transposes x and the weights so
every on-device matmul consumes operands in natural layout (zero on-device
transposes); the output projection partial sums are pair-reduced on host (+bias).

Per-core pipeline (all matmuls fp32r = FP22 multiply, fp32 PSUM accumulate):
  qT[o,t] = Wq_g @ x^T        (lhsT = Wq_g^T chunks, rhs = x^T chunks)
  kT[o,t] likewise; v[t,o]    (lhsT = x^T chunks, rhs = Wv_g^T)
  scores^T[k,q] per head      (lhsT = kT tile [64,128], rhs = qT tile [64,512])
  p = exp(0.125*scores^T)     (ACT, causal mask via memset + triangle multiply)
  [AV^T | denom] = [v|1]^T @ p (ones column of v gives softmax denominators)
  avT = AV^T * exp(-ln(denom)) broadcast via K=1 ones-matmul
  out_partial = avT^T @ Wp_g^T
"""

import numpy as np

import concourse.bass as bass
import concourse.mybir as mybir
import concourse.tile as tile
from concourse.bass_utils import run_bass_kernel_spmd

F32 = mybir.dt.float32
F32R = mybir.dt.float32r
AF = mybir.ActivationFunctionType
MULT = mybir.AluOpType.mult

B, T, C = 4, 2048, 1024
HEADS, D = 16, 64
GROUPS = 2                  # head groups (tensor parallel)
HPC = HEADS // GROUPS       # heads per core = 8
GC = HPC * D                # group channel width = 512
NKC = T // 128              # Tk chunks = 16
NJ = T // 512               # Tq tiles = 4
CCH = C // 128              # contraction chunks = 8
NSTRIP = T // 512           # phase-1 t strips = 4

_PROGRAM = None


def _patch_drain_chunking():
    """The axon walrus build rejects instructions with >~4 sem waits; Tile's
    kernel-tail drain waits on every live semaphore at once. Split it into a
    chain of drains with <=2 waits each."""
    from bass_rust import VectorClock, ScopedClock

    if getattr(tile.TileContext, "_drain_chunk_patched", False):
        return

    def _drain_and_barrier(self, tick_clock, wait_clock):
        gc_vec = list(tick_clock.global_clock)
        nz = [i for i, t in enumerate(gc_vec) if t > 0]
        CHUNK = 1
        for k in range(0, len(nz), CHUNK):
            keep = set(nz[k:k + CHUNK])
            partial = [gc_vec[i] if i in keep else 0 for i in range(len(gc_vec))]
            d = self.nc.sync.drain()
            wait_clock.add_sem_waits(d.ins, ScopedClock({None: VectorClock(partial)}))
        self.nc.all_engine_barrier()
        assert self.sems is not None
        popped = self.nc._tile_sem_poison_stack.pop()
        assert popped is self._sem_poison
        self.nc.clear_and_free_semaphores(list(self.sems.allocated().values()))
        self.nc.all_engine_barrier()

    tile.TileContext._drain_and_barrier = _drain_and_barrier
    tile.TileContext._drain_chunk_patched = True


def _split_excess_waits(nc, maxw=1, maxw_other=None):
    """Walrus rejects instructions carrying more than ~1 sem wait (proven for
    PE matmul S3_LW and the SP drain at 5). Move excess waits onto same-engine
    NoOps inserted immediately before the instruction (engine streams execute
    in bb order, so semantics are preserved). maxw_other, if set, applies to
    non-PE engines."""
    from bass_rust import InstNoOp

    ctr = 0
    for f in nc.m.functions:
        for bb in f.blocks:
            new_insts = []
            for inst in bb.instructions:
                si = inst.sync_info
                waits = list(si.on_wait) if si and si.on_wait else []
                lim = maxw
                if maxw_other is not None and str(inst.engine) != 'EngineType.PE':
                    lim = maxw_other
                maxw_eff = lim
                if len(waits) > maxw_eff:
                    head, rest = waits[:-maxw_eff], waits[-maxw_eff:]
                    for k in range(0, len(head), maxw_eff):
                        ctr += 1
                        new_insts.append(InstNoOp(
                            name=f"waitnop_{ctr}",
                            engine=inst.engine,
                            sync_info=mybir.SyncInfo(
                                on_wait=head[k:k + maxw_eff], on_update=[]),
                        ))
                    inst.sync_info = mybir.SyncInfo(on_wait=rest, on_update=si.on_update)
                new_insts.append(inst)
            bb.instructions = new_insts
    return ctr


def _build_program():
    _patch_drain_chunking()
    nc = bass.Bass()

    xT_d = nc.declare_dram_parameter("xT", [C, T], F32R, isOutput=False)
    wq_d = nc.declare_dram_parameter("wqT", [C, GC], F32R, isOutput=False)
    wk_d = nc.declare_dram_parameter("wkT", [C, GC], F32R, isOutput=False)
    wv_d = nc.declare_dram_parameter("wvT", [C, GC], F32R, isOutput=False)
    wp_d = nc.declare_dram_parameter("wpT", [GC, C], F32R, isOutput=False)
    out_d = nc.declare_dram_parameter("outp", [T, C], F32, isOutput=True)

    from contextlib import ExitStack

    with tile.TileContext(nc) as tc, ExitStack() as stack:
        cpool = stack.enter_context(tc.tile_pool(name="const", bufs=1))
        qkv_pool = stack.enter_context(tc.tile_pool(name="qkv", bufs=1))

        # additive causal mask: 0 where q >= k, -1e9 where q < k (exp -> 0)
        maskneg = cpool.tile([128, 128], F32)
        nc.gpsimd.memset(maskneg[:, :], 0.0)
        nc.gpsimd.affine_select(
            out=maskneg[:, :], in_=maskneg[:, :],
            compare_op=mybir.AluOpType.is_ge, fill=-1e9, base=0,
            pattern=[[1, 128]], channel_multiplier=-1,
        )
        # DVE cannot encode f32r, so f32r tiles are written by ACT/DMA only
        ones = cpool.tile([128, 128], F32R)
        nc.scalar.activation(ones[64:65, :], ones[64:65, :], AF.Copy, scale=0.0, bias=1.0)

        qT = qkv_pool.tile([128, HPC // 2, T], F32R)   # [c, head-pair, t]
        kT = qkv_pool.tile([128, HPC // 2, T], F32R)
        # v padded with a ones column per head: [t-chunk, head, 65]
        v = qkv_pool.tile([128, NKC, HPC, D + 1], F32R)
        nc.scalar.activation(v[:, :, :, D:D + 1], v[:, :, :, D:D + 1],
                             AF.Copy, scale=0.0, bias=1.0)

        # ---------------- Phase 1: QKV projections ----------------
        with tc.tile_pool(name="w1", bufs=1) as wpool, \
             tc.tile_pool(name="xs", bufs=3) as xpool, \
             tc.tile_pool(name="tmp1", bufs=4) as tmp1, \
             tc.tile_pool(name="ps1", bufs=8, space="PSUM") as ps1:
            wq = wpool.tile([128, CCH, GC], F32R)
            wk = wpool.tile([128, CCH, GC], F32R)
            wv = wpool.tile([128, CCH, GC], F32R)

            for s in range(NSTRIP):
                xs = xpool.tile([128, CCH, 512], F32R)
                nc.sync.dma_start(
                    xs[:, :, :],
                    xT_d[:, 512 * s:512 * (s + 1)].rearrange("(c p) t -> p c t", p=128))
                if s == 0:
                    # batched weight loads, emitted after the first x strip so
                    # the PE can start as soon as wq lands (wq first: q runs first)
                    for w_sb, w_d in ((wq, wq_d), (wk, wk_d), (wv, wv_d)):
                        nc.sync.dma_start(w_sb[:, :, :],
                                          w_d[:, :].rearrange("(c p) o -> p c o", p=128))
                for w_sb, dst in ((wq, qT), (wk, kT)):
                    for o in range(HPC // 2):
                        pq = ps1.tile([128, 512], F32, tag="pp")
                        for c in range(CCH):
                            nc.tensor.matmul(pq[:, :], w_sb[:, c, 128 * o:128 * (o + 1)],
                                             xs[:, c, :], start=(c == 0), stop=(c == CCH - 1))
                        tq = tmp1.tile([128, 512], F32, tag="t1")
                        nc.vector.tensor_copy(tq[:, :], pq[:, :])
                        nc.sync.dma_start(dst[:, o, 512 * s:512 * (s + 1)],
                                          tq[:, :].bitcast(F32R))
                for tt in range(4):
                    pv = ps1.tile([128, 512], F32, tag="pp")
                    for c in range(CCH):
                        nc.tensor.matmul(pv[:, :], xs[:, c, 128 * tt:128 * (tt + 1)],
                                         wv[:, c, :], start=(c == 0), stop=(c == CCH - 1))
                    tv = tmp1.tile([128, 512], F32, tag="t1")
                    nc.vector.tensor_copy(tv[:, :], pv[:, :])
                    nc.sync.dma_start(
                        v[:, 4 * s + tt, :, 0:D],
                        tv[:, :].rearrange("p (h d) -> p h d", h=HPC).bitcast(F32R))

        # ---------------- Phase 2+3: attention + output projection ----------------
        avT = stack.enter_context(tc.tile_pool(name="avt", bufs=1)).tile([128, HPC // 2, T], F32R)
        wp = stack.enter_context(tc.tile_pool(name="wp", bufs=1)).tile([128, GC // 128, C], F32R)
        nc.sync.dma_start(wp[:, :, :], wp_d[:, :].rearrange("(c p) o -> p c o", p=128))

        with tc.tile_pool(name="pt", bufs=8) as pt_pool, \
             tc.tile_pool(name="dd", bufs=4) as d_pool, \
             tc.tile_pool(name="rr", bufs=3) as r_pool, \
             tc.tile_pool(name="avtmp", bufs=3) as avtmp_pool, \
             tc.tile_pool(name="ob", bufs=4) as out_pool, \
             tc.tile_pool(name="ps_s", bufs=4, space="PSUM") as ps_s, \
             tc.tile_pool(name="ps_av", bufs=2, space="PSUM") as ps_av, \
             tc.tile_pool(name="ps_bc", bufs=1, space="PSUM") as ps_bc, \
             tc.tile_pool(name="ps_o", bufs=1, space="PSUM") as ps_o:

            for j in range(NJ):
                for hp in range(HPC // 2):
                    nkc = 4 * (j + 1)
                    # both heads of the pair run interleaved: their scores
                    # matmuls sit in adjacent PE slots with disjoint row
                    # groups (K=64 at partition 0 vs 64) and overlap on HW
                    av0 = ps_av.tile([65, 512], F32, tag="av")
                    av1 = ps_av.tile([65, 512], F32, tag="av")
                    avs = [av0, av1]
                    def emit_scores_exp(i):
                        out = []
                        for par in range(2):
                            pb = 64 * par
                            sps = ps_s.tile([128, 512], F32, tag="s")
                            nc.tensor.matmul(
                                sps[:, :],
                                kT[pb:pb + 64, hp, 128 * i:128 * (i + 1)],
                                qT[pb:pb + 64, hp, 512 * j:512 * (j + 1)],
                                start=True, stop=True)
                            ptile = pt_pool.tile([128, 512], F32R, tag="pt")
                            roff = 128 * i - 512 * j
                            if roff >= 0:
                                # diagonal tile: add -1e9 above the diagonal in
                                # PSUM, then exp only the columns [roff:512] the
                                # AV matmul will consume (cols [0:roff] are
                                # fully masked and skipped outright)
                                nc.vector.tensor_tensor(
                                    sps[:, roff:roff + 128], sps[:, roff:roff + 128],
                                    maskneg[:, :], op=mybir.AluOpType.add)
                                nc.scalar.activation(ptile[:, roff:512], sps[:, roff:512],
                                                     AF.Exp, scale=0.125)
                            else:
                                roff = 0
                                nc.scalar.activation(ptile[:, :], sps[:, :], AF.Exp, scale=0.125)
                            out.append((ptile, roff))
                        return out

                    def emit_av(i, pts):
                        for par in range(2):
                            ptile, roff = pts[par]
                            nc.tensor.matmul(avs[par][:, roff:512], v[:, i, 2 * hp + par, :],
                                             ptile[:, roff:512],
                                             start=(i == 0), stop=(i == nkc - 1))

                    # one-chunk software pipeline: chunk i+1's scores sit ahead
                    # of chunk i's AV matmuls in the PE stream, so AV never
                    # waits out the exp latency
                    prev = emit_scores_exp(0)
                    for i in range(1, nkc):
                        cur = emit_scores_exp(i)
                        emit_av(i - 1, prev)
                        prev = cur
                    emit_av(nkc - 1, prev)
                    for par in range(2):
                        av = avs[par]
                        # single DVE copy frees the AV PSUM bank immediately so
                        # the next head pair's AV matmuls are not gated on the
                        # whole normalize chain
                        avr = avtmp_pool.tile([65, 512], F32, tag="avr")
                        nc.vector.tensor_copy(avr[:, :], av[:, :])
                        # softmax denominators: r = exp(-ln(denom)), broadcast via K=1 matmul
                        dt_ = d_pool.tile([65, 512], F32R, tag="d")
                        nc.scalar.activation(dt_[64:65, :], avr[64:65, :], AF.Ln)
                        nc.scalar.activation(dt_[64:65, :], dt_[64:65, :], AF.Exp, scale=-1.0)
                        bc = ps_bc.tile([128, 512], F32, tag="bc")
                        nc.tensor.matmul(bc[:, :], ones[64:65, :], dt_[64:65, :],
                                         start=True, stop=True)
                        rb = r_pool.tile([64, 512], F32, tag="r")
                        nc.vector.tensor_copy(rb[:, :], bc[0:64, :])
                        avf = avtmp_pool.tile([64, 512], F32, tag="avf")
                        nc.vector.tensor_tensor(avf[:, :], avr[0:64, :], rb[:, :], op=MULT)
                        # DMA moves lanes 0:64 to the destination partitions
                        nc.sync.dma_start(avT[64 * par:64 * par + 64, hp, 512 * j:512 * (j + 1)],
                                          avf[:, :].bitcast(F32R))

                # output projection for the t-tiles whose avT columns just completed
                for tt in range(4 * j, 4 * (j + 1)):
                    ob = out_pool.tile([128, C], F32, tag="ob")
                    for o2 in range(2):
                        po = ps_o.tile([128, 512], F32, tag="o")
                        for c4 in range(GC // 128):
                            nc.tensor.matmul(po[:, :], avT[:, c4, 128 * tt:128 * (tt + 1)],
                                             wp[:, c4, 512 * o2:512 * (o2 + 1)],
                                             start=(c4 == 0), stop=(c4 == GC // 128 - 1))
                        nc.vector.tensor_copy(ob[:, 512 * o2:512 * (o2 + 1)], po[:, :])
                    nc.sync.dma_start(out_d[128 * tt:128 * (tt + 1), :], ob[:, :])
    _split_excess_waits(nc)
    return nc


def _get_program():
    global _PROGRAM
    if _PROGRAM is None:
        _PROGRAM = _build_program()
    return _PROGRAM


def _make_in_maps(x, Wk, Wq, Wv, Wp):
    x = np.asarray(x, dtype=np.float32)
    Wk = np.asarray(Wk, dtype=np.float32)
    Wq = np.asarray(Wq, dtype=np.float32)
    Wv = np.asarray(Wv, dtype=np.float32)
    Wp = np.asarray(Wp, dtype=np.float32)
    in_maps = []
    for core in range(8):
        b, g = core // GROUPS, core % GROUPS
        rows = slice(GC * g, GC * (g + 1))
        in_maps.append({
            "xT": np.ascontiguousarray(x[b].T),                 # [C, T]
            "wqT": np.ascontiguousarray(Wq[rows, :].T),         # [C, GC]
            "wkT": np.ascontiguousarray(Wk[rows, :].T),
            "wvT": np.ascontiguousarray(Wv[rows, :].T),
            "wpT": np.ascontiguousarray(Wp[:, rows].T),         # [GC, C]
        })
    return in_maps


def run(x, Wk, Wq, Wv, Wp, bp, trace=False, **spmd_kwargs):
    nc = _get_program()
    in_maps = _make_in_maps(x, Wk, Wq, Wv, Wp)
    res = run_bass_kernel_spmd(nc, in_maps, list(range(8)), trace=trace, **spmd_kwargs)
    bp = np.asarray(bp, dtype=np.float32)
    out = np.empty((B, T, C), dtype=np.float32)
    for b in range(B):
        out[b] = res.results[GROUPS * b]["outp"] + res.results[GROUPS * b + 1]["outp"] + bp
    return out, res


def kernel(x, Wk, Wq, Wv, Wp, bp):
    out, _ = run(x, Wk, Wq, Wv, Wp, bp)
    return out
